# revision 50
# baseline (speedup 1.0000x reference)
import math
import os
import numpy as np
from contextlib import ExitStack

import concourse.bass as bass
import concourse.tile as tile
from concourse import bacc, mybir
from concourse.bass_utils import run_bass_kernel_spmd
from concourse.masks import make_identity

f32 = mybir.dt.float32
f32r = mybir.dt.float32r
bf16 = mybir.dt.bfloat16
i32 = mybir.dt.int32
AF = mybir.ActivationFunctionType
OP = mybir.AluOpType

V, D, H, L, F = 32000, 768, 12, 6, 3072
B, S = 2, 1024
P = 128
DK = 64
DCH = D // P
FCH = F // P
TOK = 256
NCORE, GRP = 8, 4
VCHK = 500
VBLK = 2000
NVB = V // VBLK
EPS = 1e-5
SQD = math.sqrt(D)
ISQDK = 1.0 / math.sqrt(DK)
VW = DK + 1
VROW = 2 * H * VW

BQ, BK, BV, BO, B2, G1, BE1, G2, BE2, B1 = 0, 6, 12, 18, 24, 30, 36, 42, 48, 54


def build(nc):
    def din(name, shape, dt=f32):
        return nc.dram_tensor(name, shape, dt, kind="ExternalInput").ap()

    tok = din("tok", [P, 2], i32)
    emb = din("emb", [V, D])
    peT = din("peT", [D, TOK])
    maskt = din("maskt", [8 * P, 2 * TOK])
    wq = din("wq", [L, P, DCH * D], bf16)
    wk = din("wk", [L, P, DCH * D], bf16)
    wv = din("wv", [L, P, DCH * D], bf16)
    wo = din("wo", [L, P, DCH * D], bf16)
    w1 = din("w1", [L, P, DCH * F], bf16)
    w2 = din("w2", [L, P, FCH * D], bf16)
    ball = din("ball", [L, P, 78])
    gfp = din("gfp", [P, DCH])
    bfp = din("bfp", [P, DCH])
    woutc = din("woutc", [D, V], bf16)

    out = nc.dram_tensor("out", [TOK, V], f32, kind="ExternalOutput").ap()

    VH = P * VROW // 2
    kins = [nc.dram_tensor(f"kin{l}", [D * TOK], bf16).ap() for l in range(L)]
    kouts = [nc.dram_tensor(f"kout{l}", [GRP * D * TOK], bf16).ap()
             for l in range(L)]
    vins = [[nc.dram_tensor(f"vin{l}_{p}", [VH], bf16).ap() for p in range(2)]
            for l in range(L)]
    vouts = [[nc.dram_tensor(f"vout{l}_{p}", [GRP * VH], bf16).ap()
              for p in range(2)] for l in range(L)]
    pr4i = nc.dram_tensor("pr4i", [P], f32).ap()
    pr4o = nc.dram_tensor("pr4o", [GRP * P], f32).ap()

    KV_GROUPS = [[0, 1, 2, 3], [4, 5, 6, 7]]

    with tile.TileContext(
            nc, trace_sim=os.environ.get("TRACE_SIM", "0") == "1",
    ) as tc, ExitStack() as octx, \
            nc.allow_low_precision(reason="bf16 matmul inputs, fp32 accumulate"):
        const = octx.enter_context(tc.tile_pool(name="const", bufs=1))
        acts = octx.enter_context(tc.tile_pool(name="acts", bufs=2))
        stats = octx.enter_context(tc.tile_pool(name="stats", bufs=6))
        psum = octx.enter_context(tc.tile_pool(name="psum", bufs=8, space="PSUM"))

        nc.gpsimd.collective_compute(
            "AllGather", OP.bypass, replica_groups=KV_GROUPS,
            ins=[pr4i.opt()], outs=[pr4o.opt()],
        )

        def ctile(shape, dt, nm):
            return const.tile(shape, dt, name=nm, tag=nm)

        ident = ctile([P, P], f32, "ident")
        make_identity(nc, ident[:])
        ident_b = ctile([P, P], bf16, "ident_b")
        nc.vector.tensor_copy(ident_b[:], ident[:])
        ones_col_f = ctile([P, 1], f32, "ones_col_f")
        nc.vector.memset(ones_col_f[:], 1.0)
        ones_col = ctile([P, 1], f32r, "ones_col")
        nc.vector.tensor_copy(ones_col[:], ones_col_f[:])
        ones_row_f = ctile([1, P], f32, "ones_row_f")
        nc.vector.memset(ones_row_f[:], 1.0)
        ones_row = ctile([1, P], f32r, "ones_row")
        nc.vector.tensor_copy(ones_row[:], ones_row_f[:])
        eps_t = ctile([1, 1], f32, "eps_t")
        nc.vector.memset(eps_t[:], EPS)

        tokt = ctile([P, 2], i32, "tokt")
        nc.sync.dma_start(tokt[:], tok[:])

        mtiles = []
        for kb in range(8):
            mt = ctile([P, 2 * TOK], f32, f"mask{kb}")
            nc.sync.dma_start(mt[:], maskt[kb * P : (kb + 1) * P, :])
            mtiles.append(mt)

        gft = ctile([P, DCH], f32, "gft")
        nc.sync.dma_start(gft[:], gfp[:])
        bft = ctile([P, DCH], f32, "bft")
        nc.sync.dma_start(bft[:], bfp[:])

        x = [acts.tile([P, TOK], f32r, name=f"x{d}", tag="x", bufs=8)
             for d in range(DCH)]
        xb = [acts.tile([P, TOK], bf16, name=f"xb{d}", tag="xb", bufs=8)
              for d in range(DCH)]
        with ExitStack() as ectx:
            epool = ectx.enter_context(tc.tile_pool(name="epool", bufs=2))
            for j in range(2):
                g = epool.tile([P, D], f32, name="embrow", tag="embrow", bufs=2)
                nc.gpsimd.indirect_dma_start(
                    out=g[:],
                    out_offset=None,
                    in_=emb[:],
                    in_offset=bass.IndirectOffsetOnAxis(ap=tokt[:, j : j + 1], axis=0),
                )
                for dd in range(DCH):
                    pt = epool.tile([P, P], f32, name="pe", tag="pe", bufs=3)
                    nc.sync.dma_start(
                        pt[:], peT[dd * P : (dd + 1) * P, j * P : (j + 1) * P]
                    )
                    ps = psum.tile([P, P], f32, name="tpose", tag="ps")
                    nc.tensor.transpose(ps[:], g[:, dd * P : (dd + 1) * P], ident[:])
                    sl = x[dd][:, j * P : (j + 1) * P]
                    nc.vector.tensor_scalar_mul(sl, ps[:], SQD)
                    nc.vector.tensor_add(sl, sl, pt[:])
        for dd in range(DCH):
            nc.vector.tensor_copy(xb[dd][:], x[dd][:])

        def linear_T(in_b, w_tile, bias_ap, bias_col, out_tile, copy_eng):
            outs = [psum.tile([P, TOK], f32, name=f"lps{oc}", tag="ps")
                    for oc in range(DCH)]
            for ic in range(DCH):
                for oc in range(DCH):
                    nc.tensor.matmul(
                        outs[oc][:],
                        lhsT=w_tile[:, ic * D + oc * P : ic * D + (oc + 1) * P],
                        rhs=in_b[ic][:],
                        start=(ic == 0),
                        stop=(ic == DCH - 1),
                    )
            for oc in range(DCH):
                dst = out_tile[:, oc * TOK : (oc + 1) * TOK]
                bcol = bias_ap[:, bias_col + oc : bias_col + oc + 1]
                if copy_eng == "act":
                    nc.scalar.activation(dst, outs[oc][:], AF.Identity,
                                         bias=bcol)
                else:
                    nc.vector.tensor_scalar_add(dst, outs[oc][:], bcol)

        def ln_T(in_tiles, g_ap, gcol, be_ap, becol, outname, out_tiles=None,
                 outb_tiles=None):
            st_sum = psum.tile([1, TOK], f32, name="lnsum", tag="ps")
            for dd in range(DCH):
                nc.tensor.matmul(
                    st_sum[:], lhsT=ones_col[:], rhs=in_tiles[dd][:],
                    start=(dd == 0), stop=(dd == DCH - 1),
                )
            sqs = []
            for dd in range(DCH):
                sq = acts.tile([P, TOK], f32r, name="lnsq", tag="sq", bufs=4)
                nc.scalar.activation(sq[:], in_tiles[dd][:], AF.Square)
                sqs.append(sq)
            st_sq = psum.tile([1, TOK], f32, name="lnsq2", tag="ps")
            for dd in range(DCH):
                nc.tensor.matmul(
                    st_sq[:], lhsT=ones_col[:], rhs=sqs[dd][:],
                    start=(dd == 0), stop=(dd == DCH - 1),
                )
            nm = stats.tile([1, TOK], f32r, name="nm", tag="st")
            nc.vector.tensor_scalar_mul(nm[:], st_sum[:], -1.0 / D)
            ex2 = stats.tile([1, TOK], f32, name="ex2", tag="st")
            nc.vector.tensor_scalar_mul(ex2[:], st_sq[:], 1.0 / D)
            m2 = stats.tile([1, TOK], f32, name="m2", tag="st")
            nc.vector.tensor_mul(m2[:], nm[:], nm[:])
            var = stats.tile([1, TOK], f32, name="var", tag="st")
            nc.vector.tensor_sub(var[:], ex2[:], m2[:])
            std = stats.tile([1, TOK], f32, name="std", tag="st")
            nc.scalar.activation(std[:], var[:], AF.Sqrt, bias=eps_t[:])
            rstd = stats.tile([1, TOK], f32r, name="rstd", tag="st")
            nc.vector.reciprocal(rstd[:], std[:])
            nmb = psum.tile([P, TOK], f32, name="nmb", tag="ps")
            nc.tensor.matmul(nmb[:], lhsT=ones_row[:], rhs=nm[:],
                             start=True, stop=True)
            rsb = psum.tile([P, TOK], f32, name="rsb", tag="ps")
            nc.tensor.matmul(rsb[:], lhsT=ones_row[:], rhs=rstd[:],
                             start=True, stop=True)
            outs = []
            for dd in range(DCH):
                if out_tiles is None:
                    o = acts.tile([P, TOK], f32r, name=f"{outname}{dd}",
                                  tag="x2", bufs=8)
                else:
                    o = out_tiles[dd]
                nc.vector.tensor_add(o[:], in_tiles[dd][:], nmb[:])
                nc.vector.tensor_mul(o[:], o[:], rsb[:])
                nc.vector.tensor_scalar(
                    o[:], o[:], g_ap[:, gcol + dd : gcol + dd + 1],
                    be_ap[:, becol + dd : becol + dd + 1],
                    op0=OP.mult, op1=OP.add,
                )
                if outb_tiles is not None:
                    nc.gpsimd.tensor_copy(outb_tiles[dd][:], o[:])
                outs.append(o)
            return outs

        with ExitStack() as lctx:
            wpool = lctx.enter_context(tc.tile_pool(name="wpool", bufs=2))
            wfpool = lctx.enter_context(tc.tile_pool(name="wfpool", bufs=1))
            bpool = lctx.enter_context(tc.tile_pool(name="bpool", bufs=3))
            ktall = lctx.enter_context(tc.tile_pool(name="ktall", bufs=5))
            vall = lctx.enter_context(tc.tile_pool(name="vall", bufs=5))
            expp = lctx.enter_context(tc.tile_pool(name="expp", bufs=6))
            ffp = lctx.enter_context(tc.tile_pool(name="ffp", bufs=3))
            qkvp = lctx.enter_context(tc.tile_pool(name="qkvp", bufs=2))
            opp = lctx.enter_context(tc.tile_pool(name="opp", bufs=2))

            for l in range(L):
                ballt = bpool.tile([P, 78], f32, name="ballt", tag="b", bufs=3)
                nc.sync.dma_start(ballt[:], ball[l])

                wkt = wpool.tile([P, DCH * D], bf16, name="wkt", tag="wk", bufs=2)
                nc.gpsimd.dma_start(wkt[:], wk[l])
                wvt = wpool.tile([P, DCH * D], bf16, name="wvt", tag="wv", bufs=1)
                nc.gpsimd.dma_start(wvt[:], wv[l])
                wqt = wpool.tile([P, DCH * D], bf16, name="wqt", tag="wq", bufs=1)
                nc.gpsimd.dma_start(wqt[:], wq[l])
                wot = wpool.tile([P, DCH * D], bf16, name="wot", tag="wo", bufs=1)
                nc.gpsimd.dma_start(wot[:], wo[l])
                w1t = wfpool.tile([P, DCH * F], bf16, name="w1t", tag="w1", bufs=1)
                nc.gpsimd.dma_start(w1t[:], w1[l])

                VHC = VROW // 2
                kTt = qkvp.tile([P, DCH * TOK], bf16, name="kTt", tag="k", bufs=1)
                linear_T(xb, wkt, ballt, BK, kTt, "dve")
                nc.sync.dma_start(
                    kins[l].rearrange("(p c) -> p c", p=P), kTt[:])
                nc.gpsimd.collective_compute(
                    "AllGather", OP.bypass, replica_groups=KV_GROUPS,
                    ins=[kins[l].opt()], outs=[kouts[l].opt()],
                )

                vTt = qkvp.tile([P, DCH * TOK], bf16, name="vTt", tag="v", bufs=1)
                linear_T(xb, wvt, ballt, BV, vTt, "act")
                vns = []
                for piece in range(2):
                    vn = vall.tile([P, VHC], bf16, name=f"vn{piece}",
                                   tag="vn", bufs=11)
                    for dd in range(3 * piece, 3 * piece + 3):
                        for tch in range(2):
                            ps = psum.tile([P, P], bf16, name="vtp", tag="ps")
                            nc.tensor.transpose(
                                ps[:],
                                vTt[:, dd * TOK + tch * P :
                                    dd * TOK + (tch + 1) * P],
                                ident_b[:],
                            )
                            for hh in range(2):
                                hm = (2 * dd + hh) % 6
                                nc.vector.tensor_copy(
                                    vn[:, tch * 6 * VW + hm * VW :
                                       tch * 6 * VW + hm * VW + DK],
                                    ps[:, hh * DK : (hh + 1) * DK],
                                )
                    for tch in range(2):
                        for hm in range(6):
                            nc.vector.memset(
                                vn[:, tch * 6 * VW + hm * VW + DK :
                                   tch * 6 * VW + hm * VW + VW], 1.0)
                    vns.append(vn)
                    nc.sync.dma_start(
                        vins[l][piece].rearrange("(p c) -> p c", p=P), vn[:])
                    nc.gpsimd.collective_compute(
                        "AllGather", OP.bypass, replica_groups=KV_GROUPS,
                        ins=[vins[l][piece].opt()],
                        outs=[vouts[l][piece].opt()],
                    )

                qTt = qkvp.tile([P, DCH * TOK], bf16, name="qTt", tag="q", bufs=1)
                linear_T(xb, wqt, ballt, BQ, qTt, "act")

                KTg = []
                Vg = [[], []]
                for g in range(GRP):
                    kt = ktall.tile([P, DCH * TOK], bf16, name=f"KT{g}",
                                    tag="kt", bufs=5)
                    nc.sync.dma_start(
                        kt[:],
                        kouts[l][g * D * TOK : (g + 1) * D * TOK]
                        .rearrange("(p c) -> p c", p=P),
                    )
                    KTg.append(kt)
                for p in range(2):
                    for g in range(GRP):
                        vg = vall.tile([P, VHC], bf16, name=f"Vg{p}_{g}",
                                       tag="vn", bufs=11)
                        nc.sync.dma_start(
                            vg[:],
                            vouts[l][p][g * VH : (g + 1) * VH]
                            .rearrange("(p c) -> p c", p=P),
                        )
                        Vg[p].append(vg)

                opair = [opp.tile([P, TOK], bf16, name=f"op{hp}", tag="op",
                                  bufs=7) for hp in range(DCH)]
                for hp in range(DCH):
                    pc = hp // 3
                    hpl = hp - 3 * pc
                    oTs = [psum.tile([VW, TOK], f32, name=f"oTps{s}", tag="ps")
                           for s in range(2)]
                    for kb in range(8):
                        g, ch = divmod(kb, 2)
                        for sub in range(2):
                            prow = sub * DK
                            sc = psum.tile([P, TOK], f32, name="scps",
                                           tag="ps")
                            nc.tensor.matmul(
                                sc[:],
                                lhsT=KTg[g][prow : prow + DK,
                                            hp * TOK + ch * P :
                                            hp * TOK + (ch + 1) * P],
                                rhs=qTt[prow : prow + DK,
                                        hp * TOK : (hp + 1) * TOK],
                                start=True,
                                stop=True,
                            )
                            e = expp.tile([P, TOK], f32r, name="epre",
                                          tag="ep", bufs=3)
                            nc.vector.scalar_tensor_tensor(
                                e[:], sc[:], ISQDK,
                                mtiles[kb][:, sub * TOK : (sub + 1) * TOK],
                                op0=OP.mult, op1=OP.add)
                            eb = expp.tile([P, TOK], bf16, name="eb",
                                           tag="eb", bufs=3)
                            nc.scalar.activation(eb[:], e[:], AF.Exp)
                            hm = (2 * hp + sub) % 6
                            nc.tensor.matmul(
                                oTs[sub][:],
                                lhsT=Vg[pc][g][:, ch * 6 * VW + hm * VW :
                                               ch * 6 * VW + (hm + 1) * VW],
                                rhs=eb[:],
                                start=(kb == 0),
                                stop=(kb == 7),
                                skip_group_check=True,
                            )
                    for sub in range(2):
                        prow = sub * DK
                        rec = stats.tile([1, TOK], f32r, name="rec", tag="st")
                        nc.vector.reciprocal(rec[:], oTs[sub][DK : DK + 1, :])
                        rb = psum.tile([DK, TOK], f32, name="rbps", tag="ps")
                        nc.tensor.matmul(
                            rb[:], lhsT=ones_row[:, 0:DK], rhs=rec[:],
                            start=True, stop=True, skip_group_check=True,
                        )
                        rbs = acts.tile([DK, TOK], f32, name="rbs", tag="rbs",
                                        bufs=3)
                        nc.vector.tensor_copy(rbs[:], rb[:])
                        nc.vector.tensor_mul(
                            opair[hp][prow : prow + DK, :],
                            oTs[sub][0:DK, :], rbs[:])

                ops_ = [psum.tile([P, TOK], f32, name=f"ops{oc}", tag="ps")
                        for oc in range(DCH)]
                for hpp in range(DCH):
                    for oc in range(DCH):
                        nc.tensor.matmul(
                            ops_[oc][:],
                            lhsT=wot[:, hpp * D + oc * P : hpp * D + (oc + 1) * P],
                            rhs=opair[hpp][:],
                            start=(hpp == 0),
                            stop=(hpp == DCH - 1),
                        )
                t1 = []
                for oc in range(DCH):
                    t = acts.tile([P, TOK], f32r, name=f"t1_{oc}", tag="t",
                                  bufs=8)
                    nc.vector.tensor_scalar_add(t[:], ops_[oc][:],
                                                ballt[:, BO + oc : BO + oc + 1])
                    nc.vector.tensor_add(t[:], t[:], x[oc][:])
                    t1.append(t)
                xn1b = [acts.tile([P, TOK], bf16, name=f"xn1b{d}", tag="xb2",
                                  bufs=8) for d in range(DCH)]
                xn1 = ln_T(t1, ballt, G1, ballt, BE1, "xn1_",
                           outb_tiles=xn1b)

                yps = [psum.tile([P, TOK], f32, name=f"yps{oc}", tag="ps")
                       for oc in range(DCH)]
                for fc0 in range(0, FCH, 2):
                    fpss = []
                    for k in range(2):
                        fpss.append(psum.tile([P, TOK], f32, name=f"ffps{k}",
                                              tag="ps"))
                    for ic in range(DCH):
                        for k in range(2):
                            fc = fc0 + k
                            nc.tensor.matmul(
                                fpss[k][:],
                                lhsT=w1t[:, ic * F + fc * P :
                                         ic * F + (fc + 1) * P],
                                rhs=xn1b[ic][:],
                                start=(ic == 0),
                                stop=(ic == DCH - 1),
                            )
                    for k in range(2):
                        fc = fc0 + k
                        ft = ffp.tile([P, TOK], bf16, name="fft", tag="ff",
                                      bufs=3)
                        if k == 0:
                            nc.scalar.activation(
                                ft[:], fpss[k][:], AF.Relu,
                                bias=ballt[:, B1 + fc : B1 + fc + 1],
                            )
                        else:
                            nc.vector.tensor_scalar(
                                ft[:], fpss[k][:],
                                ballt[:, B1 + fc : B1 + fc + 1], 0.0,
                                op0=OP.add, op1=OP.max,
                            )
                        w2c = wfpool.tile([P, D], bf16, name="w2c", tag="w2",
                                          bufs=6)
                        nc.gpsimd.dma_start(
                            w2c[:], w2[l][:, fc * D : (fc + 1) * D])
                        for oc in range(DCH):
                            nc.tensor.matmul(
                                yps[oc][:],
                                lhsT=w2c[:, oc * P : (oc + 1) * P],
                                rhs=ft[:],
                                start=(fc == 0),
                                stop=(fc == FCH - 1),
                            )
                t2 = []
                for oc in range(DCH):
                    t = acts.tile([P, TOK], f32r, name=f"t2_{oc}", tag="t",
                                  bufs=8)
                    nc.vector.tensor_scalar_add(
                        t[:], yps[oc][:],
                        ballt[:, B2 + oc : B2 + oc + 1])
                    nc.vector.tensor_add(t[:], t[:], xn1[oc][:])
                    t2.append(t)
                x = [acts.tile([P, TOK], f32r, name=f"xo{d}", tag="x",
                               bufs=8) for d in range(DCH)]
                xb = [acts.tile([P, TOK], bf16, name=f"xob{d}", tag="xb",
                                bufs=8) for d in range(DCH)]
                ln_T(t2, ballt, G2, ballt, BE2, f"xl{l}_", out_tiles=x,
                     outb_tiles=xb)

        hb = [acts.tile([P, TOK], bf16, name=f"hb{d}", tag="xb2", bufs=8)
              for d in range(DCH)]
        ln_T(x, gft, 0, bft, 0, "hT_", outb_tiles=hb)

        with ExitStack() as hctx:
            wopool = hctx.enter_context(tc.tile_pool(name="wopool", bufs=2))
            osb = hctx.enter_context(tc.tile_pool(name="osb", bufs=6))

            for vb in range(NVB):
                wts = []
                for dd in range(DCH):
                    wt = wopool.tile([P, VBLK], bf16, name="woutt",
                                     tag=f"wo{dd}", bufs=2)
                    nc.sync.dma_start(
                        wt[:],
                        woutc[dd * P : (dd + 1) * P,
                              vb * VBLK : (vb + 1) * VBLK],
                    )
                    wts.append(wt)
                for tch in range(2):
                    for vc0 in range(0, VBLK // VCHK, 2):
                        lps = [psum.tile([P, VCHK], f32, name=f"logps{k}",
                                         tag="ps") for k in range(2)]
                        for dd in range(DCH):
                            for k in range(2):
                                nc.tensor.matmul(
                                    lps[k][:],
                                    lhsT=hb[dd][:, tch * P : (tch + 1) * P],
                                    rhs=wts[dd][:, (vc0 + k) * VCHK :
                                                (vc0 + k + 1) * VCHK],
                                    start=(dd == 0),
                                    stop=(dd == DCH - 1),
                                )
                        for k in range(2):
                            v0 = vb * VBLK + (vc0 + k) * VCHK
                            ot = osb.tile([P, VCHK], f32, name="lsb",
                                          tag="lsb", bufs=6)
                            if k == 0:
                                nc.vector.tensor_copy(ot[:], lps[k][:])
                            else:
                                nc.scalar.copy(ot[:], lps[k][:])
                            nc.gpsimd.dma_start(
                                out[tch * P : (tch + 1) * P, v0 : v0 + VCHK],
                                ot[:])

    return nc


_CACHED = {}
_BOUT = {}


def _compiled():
    if "nc" not in _CACHED:
        nc = bacc.Bacc("TRN2", target_bir_lowering=False, debug=False,
                       num_devices=NCORE)
        build(nc)
        nc.compile()
        _CACHED["nc"] = nc
    return _CACHED["nc"]


def _make_inputs(tokens, emb, pe, wq, bq, wk, bk, wv, bv, wo, bo,
                 w1, b1, w2, b2, g1, be1, g2, be2, gf, bf, wout, bout):
    import ml_dtypes
    f = np.float32
    b16 = ml_dtypes.bfloat16
    tokens = np.asarray(tokens).astype(np.int32)

    def parr(b):
        b = np.asarray(b, f)
        return b.reshape(L, b.shape[1] // P, P).transpose(0, 2, 1)

    def parr1(b):
        b = np.asarray(b, f)
        return np.ascontiguousarray(b.reshape(b.shape[0] // P, P).T)

    def pslab(w):
        w = np.asarray(w, f)
        Lc, R, C = w.shape
        return np.ascontiguousarray(
            w.reshape(Lc, R // P, P, C).transpose(0, 2, 1, 3)
            .reshape(Lc, P, (R // P) * C).astype(b16))

    ball = np.concatenate(
        [parr(bq), parr(bk), parr(bv), parr(bo), parr(b2),
         parr(g1), parr(be1), parr(g2), parr(be2), parr(b1)], axis=2)

    common = {
        "emb": np.ascontiguousarray(np.asarray(emb, f)),
        "wq": pslab(wq),
        "wk": pslab(wk),
        "wv": pslab(wv),
        "wo": pslab(wo),
        "w1": pslab(w1),
        "w2": pslab(w2),
        "ball": np.ascontiguousarray(ball),
        "gfp": parr1(gf), "bfp": parr1(bf),
        "woutc": np.ascontiguousarray(np.asarray(wout, f).astype(b16)),
    }
    _BOUT["v"] = np.asarray(bout, f)
    pe = np.asarray(pe, f)

    in_maps = []
    for c in range(NCORE):
        b, r = divmod(c, GRP)
        chunks = (r, 7 - r)
        rows = np.concatenate(
            [np.arange(ch * P, (ch + 1) * P) for ch in chunks])
        tok_c = np.stack(
            [tokens[b, ch * P : (ch + 1) * P] for ch in chunks], axis=1
        ).astype(np.int32)
        peT_c = np.ascontiguousarray(pe[rows].T)

        kpos = np.empty(8 * P, np.int64)
        for kb in range(8):
            g, chi = divmod(kb, 2)
            ch = g if chi == 0 else 7 - g
            kpos[kb * P : (kb + 1) * P] = np.arange(ch * P, (ch + 1) * P)
        qpos = rows
        mask = np.where(kpos[:, None] <= qpos[None, :], 0.0, -1e9).astype(f)
        mask2 = np.concatenate([mask, mask], axis=1)

        m = dict(common)
        m.update({
            "tok": tok_c,
            "peT": peT_c,
            "maskt": np.ascontiguousarray(mask2),
        })
        in_maps.append(m)
    return in_maps


def run(in_maps, **kwargs):
    nc = _compiled()
    return run_bass_kernel_spmd(nc, in_maps, list(range(NCORE)), **kwargs)


def assemble(results):
    full = np.empty((B, S, V), np.float32)
    bout = _BOUT["v"]
    for c in range(NCORE):
        lt = np.asarray(results[c]["out"])
        bc, rc = divmod(c, GRP)
        for hi, ch in enumerate((rc, 7 - rc)):
            full[bc, ch * P : (ch + 1) * P, :] = \
                lt[hi * P : (hi + 1) * P, :] + bout
    return full


def kernel(**inputs):
    in_maps = _make_inputs(**inputs)
    res = run(in_maps)
    return assemble(res.results)


# revision 51
# speedup vs baseline: 1.0461x; 1.0461x over previous
import math
import os
import numpy as np
from contextlib import ExitStack

import concourse.bass as bass
import concourse.tile as tile
from concourse import bacc, mybir
from concourse.bass_utils import run_bass_kernel_spmd
from concourse.masks import make_identity

f32 = mybir.dt.float32
f32r = mybir.dt.float32r
bf16 = mybir.dt.bfloat16
i32 = mybir.dt.int32
AF = mybir.ActivationFunctionType
OP = mybir.AluOpType

V, D, H, L, F = 32000, 768, 12, 6, 3072
B, S = 2, 1024
P = 128
DK = 64
DCH = D // P
FCH = F // P
TOK = 256
NCORE, GRP = 8, 4
VCHK = 500
VBLK = 2000
NVB = V // VBLK
EPS = 1e-5
SQD = math.sqrt(D)
ISQDK = 1.0 / math.sqrt(DK)
VW = DK + 1
VROW = 2 * H * VW

BQ, BK, BV, BO, B2, G1, BE1, G2, BE2, B1 = 0, 6, 12, 18, 24, 30, 36, 42, 48, 54


def build(nc):
    def din(name, shape, dt=f32):
        return nc.dram_tensor(name, shape, dt, kind="ExternalInput").ap()

    tok = din("tok", [P, 2], i32)
    emb = din("emb", [V, D])
    peT = din("peT", [D, TOK])
    maskt = din("maskt", [8 * P, 2 * TOK])
    wq = din("wq", [L, P, DCH * D], bf16)
    wk = din("wk", [L, P, DCH * D], bf16)
    wv = din("wv", [L, P, DCH * D], bf16)
    wo = din("wo", [L, P, DCH * D], bf16)
    w1 = din("w1", [L, P, DCH * F], bf16)
    w2 = din("w2", [L, P, FCH * D], bf16)
    ball = din("ball", [L, P, 78])
    gfp = din("gfp", [P, DCH])
    bfp = din("bfp", [P, DCH])
    woutc = din("woutc", [D, V], bf16)

    out = nc.dram_tensor("out", [TOK, V], f32, kind="ExternalOutput").ap()

    KH = D * TOK // 2
    VH = P * VROW // 2
    kins = [[nc.dram_tensor(f"kin{l}_{p}", [KH], bf16).ap() for p in range(2)]
            for l in range(L)]
    kouts = [[nc.dram_tensor(f"kout{l}_{p}", [GRP * KH], bf16).ap()
              for p in range(2)] for l in range(L)]
    vins = [[nc.dram_tensor(f"vin{l}_{p}", [VH], bf16).ap() for p in range(2)]
            for l in range(L)]
    vouts = [[nc.dram_tensor(f"vout{l}_{p}", [GRP * VH], bf16).ap()
              for p in range(2)] for l in range(L)]
    pr4i = nc.dram_tensor("pr4i", [P], f32).ap()
    pr4o = nc.dram_tensor("pr4o", [GRP * P], f32).ap()

    KV_GROUPS = [[0, 1, 2, 3], [4, 5, 6, 7]]

    with tile.TileContext(
            nc, trace_sim=os.environ.get("TRACE_SIM", "0") == "1",
    ) as tc, ExitStack() as octx, \
            nc.allow_low_precision(reason="bf16 matmul inputs, fp32 accumulate"):
        const = octx.enter_context(tc.tile_pool(name="const", bufs=1))
        acts = octx.enter_context(tc.tile_pool(name="acts", bufs=2))
        stats = octx.enter_context(tc.tile_pool(name="stats", bufs=6))
        psum = octx.enter_context(tc.tile_pool(name="psum", bufs=8, space="PSUM"))

        nc.gpsimd.collective_compute(
            "AllGather", OP.bypass, replica_groups=KV_GROUPS,
            ins=[pr4i.opt()], outs=[pr4o.opt()],
        )

        def ctile(shape, dt, nm):
            return const.tile(shape, dt, name=nm, tag=nm)

        ident = ctile([P, P], f32, "ident")
        make_identity(nc, ident[:])
        ident_b = ctile([P, P], bf16, "ident_b")
        nc.vector.tensor_copy(ident_b[:], ident[:])
        ones_col_f = ctile([P, 1], f32, "ones_col_f")
        nc.vector.memset(ones_col_f[:], 1.0)
        ones_col = ctile([P, 1], f32r, "ones_col")
        nc.vector.tensor_copy(ones_col[:], ones_col_f[:])
        ones_row_f = ctile([1, P], f32, "ones_row_f")
        nc.vector.memset(ones_row_f[:], 1.0)
        ones_row = ctile([1, P], f32r, "ones_row")
        nc.vector.tensor_copy(ones_row[:], ones_row_f[:])
        eps_t = ctile([1, 1], f32, "eps_t")
        nc.vector.memset(eps_t[:], EPS)

        tokt = ctile([P, 2], i32, "tokt")
        nc.sync.dma_start(tokt[:], tok[:])

        mtiles = []
        for kb in range(8):
            mt = ctile([P, 2 * TOK], f32, f"mask{kb}")
            nc.sync.dma_start(mt[:], maskt[kb * P : (kb + 1) * P, :])
            mtiles.append(mt)

        gft = ctile([P, DCH], f32, "gft")
        nc.sync.dma_start(gft[:], gfp[:])
        bft = ctile([P, DCH], f32, "bft")
        nc.sync.dma_start(bft[:], bfp[:])

        x = [acts.tile([P, TOK], f32r, name=f"x{d}", tag="x", bufs=8)
             for d in range(DCH)]
        xb = [acts.tile([P, TOK], bf16, name=f"xb{d}", tag="xb", bufs=8)
              for d in range(DCH)]
        with ExitStack() as ectx:
            epool = ectx.enter_context(tc.tile_pool(name="epool", bufs=2))
            for j in range(2):
                g = epool.tile([P, D], f32, name="embrow", tag="embrow", bufs=2)
                nc.gpsimd.indirect_dma_start(
                    out=g[:],
                    out_offset=None,
                    in_=emb[:],
                    in_offset=bass.IndirectOffsetOnAxis(ap=tokt[:, j : j + 1], axis=0),
                )
                for dd in range(DCH):
                    pt = epool.tile([P, P], f32, name="pe", tag="pe", bufs=3)
                    nc.sync.dma_start(
                        pt[:], peT[dd * P : (dd + 1) * P, j * P : (j + 1) * P]
                    )
                    ps = psum.tile([P, P], f32, name="tpose", tag="ps")
                    nc.tensor.transpose(ps[:], g[:, dd * P : (dd + 1) * P], ident[:])
                    sl = x[dd][:, j * P : (j + 1) * P]
                    nc.vector.tensor_scalar_mul(sl, ps[:], SQD)
                    nc.vector.tensor_add(sl, sl, pt[:])
        for dd in range(DCH):
            nc.vector.tensor_copy(xb[dd][:], x[dd][:])

        def linear_T(in_b, w_tile, bias_ap, bias_col, out_tile, copy_eng):
            outs = [psum.tile([P, TOK], f32, name=f"lps{oc}", tag="ps")
                    for oc in range(DCH)]
            for ic in range(DCH):
                for oc in range(DCH):
                    nc.tensor.matmul(
                        outs[oc][:],
                        lhsT=w_tile[:, ic * D + oc * P : ic * D + (oc + 1) * P],
                        rhs=in_b[ic][:],
                        start=(ic == 0),
                        stop=(ic == DCH - 1),
                    )
            for oc in range(DCH):
                dst = out_tile[:, oc * TOK : (oc + 1) * TOK]
                bcol = bias_ap[:, bias_col + oc : bias_col + oc + 1]
                if copy_eng == "act":
                    nc.scalar.activation(dst, outs[oc][:], AF.Identity,
                                         bias=bcol)
                else:
                    nc.vector.tensor_scalar_add(dst, outs[oc][:], bcol)

        def ln_T(in_tiles, g_ap, gcol, be_ap, becol, outname, out_tiles=None,
                 outb_tiles=None):
            st_sum = psum.tile([1, TOK], f32, name="lnsum", tag="ps")
            for dd in range(DCH):
                nc.tensor.matmul(
                    st_sum[:], lhsT=ones_col[:], rhs=in_tiles[dd][:],
                    start=(dd == 0), stop=(dd == DCH - 1),
                )
            sqs = []
            for dd in range(DCH):
                sq = acts.tile([P, TOK], f32r, name="lnsq", tag="sq", bufs=4)
                nc.scalar.activation(sq[:], in_tiles[dd][:], AF.Square)
                sqs.append(sq)
            st_sq = psum.tile([1, TOK], f32, name="lnsq2", tag="ps")
            for dd in range(DCH):
                nc.tensor.matmul(
                    st_sq[:], lhsT=ones_col[:], rhs=sqs[dd][:],
                    start=(dd == 0), stop=(dd == DCH - 1),
                )
            nm = stats.tile([1, TOK], f32r, name="nm", tag="st")
            nc.vector.tensor_scalar_mul(nm[:], st_sum[:], -1.0 / D)
            ex2 = stats.tile([1, TOK], f32, name="ex2", tag="st")
            nc.vector.tensor_scalar_mul(ex2[:], st_sq[:], 1.0 / D)
            m2 = stats.tile([1, TOK], f32, name="m2", tag="st")
            nc.vector.tensor_mul(m2[:], nm[:], nm[:])
            var = stats.tile([1, TOK], f32, name="var", tag="st")
            nc.vector.tensor_sub(var[:], ex2[:], m2[:])
            std = stats.tile([1, TOK], f32, name="std", tag="st")
            nc.scalar.activation(std[:], var[:], AF.Sqrt, bias=eps_t[:])
            rstd = stats.tile([1, TOK], f32r, name="rstd", tag="st")
            nc.vector.reciprocal(rstd[:], std[:])
            nmb = psum.tile([P, TOK], f32, name="nmb", tag="ps")
            nc.tensor.matmul(nmb[:], lhsT=ones_row[:], rhs=nm[:],
                             start=True, stop=True)
            rsb = psum.tile([P, TOK], f32, name="rsb", tag="ps")
            nc.tensor.matmul(rsb[:], lhsT=ones_row[:], rhs=rstd[:],
                             start=True, stop=True)
            outs = []
            for dd in range(DCH):
                if out_tiles is None:
                    o = acts.tile([P, TOK], f32r, name=f"{outname}{dd}",
                                  tag="x2", bufs=8)
                else:
                    o = out_tiles[dd]
                nc.vector.tensor_add(o[:], in_tiles[dd][:], nmb[:])
                nc.vector.tensor_mul(o[:], o[:], rsb[:])
                nc.vector.tensor_scalar(
                    o[:], o[:], g_ap[:, gcol + dd : gcol + dd + 1],
                    be_ap[:, becol + dd : becol + dd + 1],
                    op0=OP.mult, op1=OP.add,
                )
                if outb_tiles is not None:
                    nc.scalar.activation(outb_tiles[dd][:], o[:], AF.Identity)
                outs.append(o)
            return outs

        with ExitStack() as lctx:
            wpool = lctx.enter_context(tc.tile_pool(name="wpool", bufs=2))
            wfpool = lctx.enter_context(tc.tile_pool(name="wfpool", bufs=1))
            bpool = lctx.enter_context(tc.tile_pool(name="bpool", bufs=3))
            ktall = lctx.enter_context(tc.tile_pool(name="ktall", bufs=5))
            vall = lctx.enter_context(tc.tile_pool(name="vall", bufs=5))
            expp = lctx.enter_context(tc.tile_pool(name="expp", bufs=6))
            ffp = lctx.enter_context(tc.tile_pool(name="ffp", bufs=3))
            qkvp = lctx.enter_context(tc.tile_pool(name="qkvp", bufs=2))
            opp = lctx.enter_context(tc.tile_pool(name="opp", bufs=2))

            for l in range(L):
                ballt = bpool.tile([P, 78], f32, name="ballt", tag="b", bufs=3)
                nc.sync.dma_start(ballt[:], ball[l])

                wkt = wpool.tile([P, DCH * D], bf16, name="wkt", tag="wk", bufs=2)
                nc.gpsimd.dma_start(wkt[:], wk[l])
                wvt = wpool.tile([P, DCH * D], bf16, name="wvt", tag="wv", bufs=1)
                nc.gpsimd.dma_start(wvt[:], wv[l])
                wqt = wpool.tile([P, DCH * D], bf16, name="wqt", tag="wq", bufs=1)
                nc.gpsimd.dma_start(wqt[:], wq[l])
                wot = wpool.tile([P, DCH * D], bf16, name="wot", tag="wo", bufs=1)
                nc.gpsimd.dma_start(wot[:], wo[l])
                w1t = wfpool.tile([P, DCH * F], bf16, name="w1t", tag="w1", bufs=1)
                nc.gpsimd.dma_start(w1t[:], w1[l])

                KHC = DCH * TOK // 2
                VHC = VROW // 2
                kTt = qkvp.tile([P, DCH * TOK], bf16, name="kTt", tag="k", bufs=1)
                linear_T(xb, wkt, ballt, BK, kTt, "dve")
                for p in range(2):
                    nc.sync.dma_start(
                        kins[l][p].rearrange("(p c) -> p c", p=P),
                        kTt[:, p * KHC : (p + 1) * KHC])
                nc.gpsimd.collective_compute(
                    "AllGather", OP.bypass, replica_groups=KV_GROUPS,
                    ins=[kins[l][0].opt()], outs=[kouts[l][0].opt()],
                )

                vTt = qkvp.tile([P, DCH * TOK], bf16, name="vTt", tag="v", bufs=1)
                linear_T(xb, wvt, ballt, BV, vTt, "act")
                vns = []
                for piece in range(2):
                    vn = vall.tile([P, VHC], bf16, name=f"vn{piece}",
                                   tag="vn", bufs=11)
                    for dd in range(3 * piece, 3 * piece + 3):
                        for tch in range(2):
                            ps = psum.tile([P, P], bf16, name="vtp", tag="ps")
                            nc.tensor.transpose(
                                ps[:],
                                vTt[:, dd * TOK + tch * P :
                                    dd * TOK + (tch + 1) * P],
                                ident_b[:],
                            )
                            for hh in range(2):
                                hm = (2 * dd + hh) % 6
                                nc.vector.tensor_copy(
                                    vn[:, tch * 6 * VW + hm * VW :
                                       tch * 6 * VW + hm * VW + DK],
                                    ps[:, hh * DK : (hh + 1) * DK],
                                )
                    for tch in range(2):
                        for hm in range(6):
                            nc.vector.memset(
                                vn[:, tch * 6 * VW + hm * VW + DK :
                                   tch * 6 * VW + hm * VW + VW], 1.0)
                    vns.append(vn)
                    nc.sync.dma_start(
                        vins[l][piece].rearrange("(p c) -> p c", p=P), vn[:])
                    nc.gpsimd.collective_compute(
                        "AllGather", OP.bypass, replica_groups=KV_GROUPS,
                        ins=[vins[l][piece].opt()],
                        outs=[vouts[l][piece].opt()],
                    )
                    if piece == 0:
                        nc.gpsimd.collective_compute(
                            "AllGather", OP.bypass, replica_groups=KV_GROUPS,
                            ins=[kins[l][1].opt()], outs=[kouts[l][1].opt()],
                        )

                qTt = qkvp.tile([P, DCH * TOK], bf16, name="qTt", tag="q", bufs=1)
                linear_T(xb, wqt, ballt, BQ, qTt, "act")

                KTg = [[], []]
                Vg = [[], []]
                for p in range(2):
                    for g in range(GRP):
                        kt = ktall.tile([P, KHC], bf16, name=f"KT{p}_{g}",
                                        tag="kt", bufs=9)
                        nc.sync.dma_start(
                            kt[:],
                            kouts[l][p][g * KH : (g + 1) * KH]
                            .rearrange("(p c) -> p c", p=P),
                        )
                        KTg[p].append(kt)
                        vg = vall.tile([P, VHC], bf16, name=f"Vg{p}_{g}",
                                       tag="vn", bufs=11)
                        nc.sync.dma_start(
                            vg[:],
                            vouts[l][p][g * VH : (g + 1) * VH]
                            .rearrange("(p c) -> p c", p=P),
                        )
                        Vg[p].append(vg)

                opair = [opp.tile([P, TOK], bf16, name=f"op{hp}", tag="op",
                                  bufs=7) for hp in range(DCH)]
                for hp in range(DCH):
                    pc = hp // 3
                    hpl = hp - 3 * pc
                    oTs = [psum.tile([VW, TOK], f32, name=f"oTps{s}", tag="ps")
                           for s in range(2)]
                    for kb in range(8):
                        g, ch = divmod(kb, 2)
                        for sub in range(2):
                            prow = sub * DK
                            sc = psum.tile([P, TOK], f32, name="scps",
                                           tag="ps")
                            nc.tensor.matmul(
                                sc[:],
                                lhsT=KTg[pc][g][prow : prow + DK,
                                                hpl * TOK + ch * P :
                                                hpl * TOK + (ch + 1) * P],
                                rhs=qTt[prow : prow + DK,
                                        hp * TOK : (hp + 1) * TOK],
                                start=True,
                                stop=True,
                            )
                            e = expp.tile([P, TOK], f32r, name="epre",
                                          tag="ep", bufs=3)
                            nc.vector.scalar_tensor_tensor(
                                e[:], sc[:], ISQDK,
                                mtiles[kb][:, sub * TOK : (sub + 1) * TOK],
                                op0=OP.mult, op1=OP.add)
                            eb = expp.tile([P, TOK], bf16, name="eb",
                                           tag="eb", bufs=3)
                            nc.scalar.activation(eb[:], e[:], AF.Exp)
                            hm = (2 * hp + sub) % 6
                            nc.tensor.matmul(
                                oTs[sub][:],
                                lhsT=Vg[pc][g][:, ch * 6 * VW + hm * VW :
                                               ch * 6 * VW + (hm + 1) * VW],
                                rhs=eb[:],
                                start=(kb == 0),
                                stop=(kb == 7),
                                skip_group_check=True,
                            )
                    for sub in range(2):
                        prow = sub * DK
                        rec = stats.tile([1, TOK], f32r, name="rec", tag="st")
                        nc.vector.reciprocal(rec[:], oTs[sub][DK : DK + 1, :])
                        rb = psum.tile([DK, TOK], f32, name="rbps", tag="ps")
                        nc.tensor.matmul(
                            rb[:], lhsT=ones_row[:, 0:DK], rhs=rec[:],
                            start=True, stop=True, skip_group_check=True,
                        )
                        rbs = acts.tile([DK, TOK], f32, name="rbs", tag="rbs",
                                        bufs=3)
                        nc.vector.tensor_copy(rbs[:], rb[:])
                        nc.vector.tensor_mul(
                            opair[hp][prow : prow + DK, :],
                            oTs[sub][0:DK, :], rbs[:])

                ops_ = [psum.tile([P, TOK], f32, name=f"ops{oc}", tag="ps")
                        for oc in range(DCH)]
                for hpp in range(DCH):
                    for oc in range(DCH):
                        nc.tensor.matmul(
                            ops_[oc][:],
                            lhsT=wot[:, hpp * D + oc * P : hpp * D + (oc + 1) * P],
                            rhs=opair[hpp][:],
                            start=(hpp == 0),
                            stop=(hpp == DCH - 1),
                        )
                t1 = []
                for oc in range(DCH):
                    t = acts.tile([P, TOK], f32r, name=f"t1_{oc}", tag="t",
                                  bufs=8)
                    nc.vector.tensor_scalar_add(t[:], ops_[oc][:],
                                                ballt[:, BO + oc : BO + oc + 1])
                    nc.vector.tensor_add(t[:], t[:], x[oc][:])
                    t1.append(t)
                xn1b = [acts.tile([P, TOK], bf16, name=f"xn1b{d}", tag="xb2",
                                  bufs=8) for d in range(DCH)]
                xn1 = ln_T(t1, ballt, G1, ballt, BE1, "xn1_",
                           outb_tiles=xn1b)

                yps = [psum.tile([P, TOK], f32, name=f"yps{oc}", tag="ps")
                       for oc in range(DCH)]
                for fc0 in range(0, FCH, 2):
                    fpss = []
                    for k in range(2):
                        fpss.append(psum.tile([P, TOK], f32, name=f"ffps{k}",
                                              tag="ps"))
                    for ic in range(DCH):
                        for k in range(2):
                            fc = fc0 + k
                            nc.tensor.matmul(
                                fpss[k][:],
                                lhsT=w1t[:, ic * F + fc * P :
                                         ic * F + (fc + 1) * P],
                                rhs=xn1b[ic][:],
                                start=(ic == 0),
                                stop=(ic == DCH - 1),
                            )
                    for k in range(2):
                        fc = fc0 + k
                        ft = ffp.tile([P, TOK], bf16, name="fft", tag="ff",
                                      bufs=3)
                        if k == 0:
                            nc.scalar.activation(
                                ft[:], fpss[k][:], AF.Relu,
                                bias=ballt[:, B1 + fc : B1 + fc + 1],
                            )
                        else:
                            nc.vector.tensor_scalar(
                                ft[:], fpss[k][:],
                                ballt[:, B1 + fc : B1 + fc + 1], 0.0,
                                op0=OP.add, op1=OP.max,
                            )
                        w2c = wfpool.tile([P, D], bf16, name="w2c", tag="w2",
                                          bufs=6)
                        nc.gpsimd.dma_start(
                            w2c[:], w2[l][:, fc * D : (fc + 1) * D])
                        for oc in range(DCH):
                            nc.tensor.matmul(
                                yps[oc][:],
                                lhsT=w2c[:, oc * P : (oc + 1) * P],
                                rhs=ft[:],
                                start=(fc == 0),
                                stop=(fc == FCH - 1),
                            )
                t2 = []
                for oc in range(DCH):
                    t = acts.tile([P, TOK], f32r, name=f"t2_{oc}", tag="t",
                                  bufs=8)
                    nc.vector.tensor_scalar_add(
                        t[:], yps[oc][:],
                        ballt[:, B2 + oc : B2 + oc + 1])
                    nc.vector.tensor_add(t[:], t[:], xn1[oc][:])
                    t2.append(t)
                x = [acts.tile([P, TOK], f32r, name=f"xo{d}", tag="x",
                               bufs=8) for d in range(DCH)]
                xb = [acts.tile([P, TOK], bf16, name=f"xob{d}", tag="xb",
                                bufs=8) for d in range(DCH)]
                ln_T(t2, ballt, G2, ballt, BE2, f"xl{l}_", out_tiles=x,
                     outb_tiles=xb)

        hb = [acts.tile([P, TOK], bf16, name=f"hb{d}", tag="xb2", bufs=8)
              for d in range(DCH)]
        ln_T(x, gft, 0, bft, 0, "hT_", outb_tiles=hb)

        with ExitStack() as hctx:
            wopool = hctx.enter_context(tc.tile_pool(name="wopool", bufs=2))
            osb = hctx.enter_context(tc.tile_pool(name="osb", bufs=6))

            for vb in range(NVB):
                wts = []
                for dd in range(DCH):
                    wt = wopool.tile([P, VBLK], bf16, name="woutt",
                                     tag=f"wo{dd}", bufs=2)
                    nc.sync.dma_start(
                        wt[:],
                        woutc[dd * P : (dd + 1) * P,
                              vb * VBLK : (vb + 1) * VBLK],
                    )
                    wts.append(wt)
                for tch in range(2):
                    for vc0 in range(0, VBLK // VCHK, 2):
                        lps = [psum.tile([P, VCHK], f32, name=f"logps{k}",
                                         tag="ps") for k in range(2)]
                        for dd in range(DCH):
                            for k in range(2):
                                nc.tensor.matmul(
                                    lps[k][:],
                                    lhsT=hb[dd][:, tch * P : (tch + 1) * P],
                                    rhs=wts[dd][:, (vc0 + k) * VCHK :
                                                (vc0 + k + 1) * VCHK],
                                    start=(dd == 0),
                                    stop=(dd == DCH - 1),
                                )
                        for k in range(2):
                            v0 = vb * VBLK + (vc0 + k) * VCHK
                            ot = osb.tile([P, VCHK], f32, name="lsb",
                                          tag="lsb", bufs=6)
                            if k == 0:
                                nc.vector.tensor_copy(ot[:], lps[k][:])
                            else:
                                nc.scalar.copy(ot[:], lps[k][:])
                            nc.gpsimd.dma_start(
                                out[tch * P : (tch + 1) * P, v0 : v0 + VCHK],
                                ot[:])

    return nc


_CACHED = {}
_BOUT = {}


def _compiled():
    if "nc" not in _CACHED:
        nc = bacc.Bacc("TRN2", target_bir_lowering=False, debug=False,
                       num_devices=NCORE)
        build(nc)
        nc.compile()
        _CACHED["nc"] = nc
    return _CACHED["nc"]


def _make_inputs(tokens, emb, pe, wq, bq, wk, bk, wv, bv, wo, bo,
                 w1, b1, w2, b2, g1, be1, g2, be2, gf, bf, wout, bout):
    import ml_dtypes
    f = np.float32
    b16 = ml_dtypes.bfloat16
    tokens = np.asarray(tokens).astype(np.int32)

    def parr(b):
        b = np.asarray(b, f)
        return b.reshape(L, b.shape[1] // P, P).transpose(0, 2, 1)

    def parr1(b):
        b = np.asarray(b, f)
        return np.ascontiguousarray(b.reshape(b.shape[0] // P, P).T)

    def pslab(w):
        w = np.asarray(w, f)
        Lc, R, C = w.shape
        return np.ascontiguousarray(
            w.reshape(Lc, R // P, P, C).transpose(0, 2, 1, 3)
            .reshape(Lc, P, (R // P) * C).astype(b16))

    ball = np.concatenate(
        [parr(bq), parr(bk), parr(bv), parr(bo), parr(b2),
         parr(g1), parr(be1), parr(g2), parr(be2), parr(b1)], axis=2)

    common = {
        "emb": np.ascontiguousarray(np.asarray(emb, f)),
        "wq": pslab(wq),
        "wk": pslab(wk),
        "wv": pslab(wv),
        "wo": pslab(wo),
        "w1": pslab(w1),
        "w2": pslab(w2),
        "ball": np.ascontiguousarray(ball),
        "gfp": parr1(gf), "bfp": parr1(bf),
        "woutc": np.ascontiguousarray(np.asarray(wout, f).astype(b16)),
    }
    _BOUT["v"] = np.asarray(bout, f)
    pe = np.asarray(pe, f)

    in_maps = []
    for c in range(NCORE):
        b, r = divmod(c, GRP)
        chunks = (r, 7 - r)
        rows = np.concatenate(
            [np.arange(ch * P, (ch + 1) * P) for ch in chunks])
        tok_c = np.stack(
            [tokens[b, ch * P : (ch + 1) * P] for ch in chunks], axis=1
        ).astype(np.int32)
        peT_c = np.ascontiguousarray(pe[rows].T)

        kpos = np.empty(8 * P, np.int64)
        for kb in range(8):
            g, chi = divmod(kb, 2)
            ch = g if chi == 0 else 7 - g
            kpos[kb * P : (kb + 1) * P] = np.arange(ch * P, (ch + 1) * P)
        qpos = rows
        mask = np.where(kpos[:, None] <= qpos[None, :], 0.0, -1e9).astype(f)
        mask2 = np.concatenate([mask, mask], axis=1)

        m = dict(common)
        m.update({
            "tok": tok_c,
            "peT": peT_c,
            "maskt": np.ascontiguousarray(mask2),
        })
        in_maps.append(m)
    return in_maps


def run(in_maps, **kwargs):
    nc = _compiled()
    return run_bass_kernel_spmd(nc, in_maps, list(range(NCORE)), **kwargs)


def assemble(results):
    full = np.empty((B, S, V), np.float32)
    bout = _BOUT["v"]
    for c in range(NCORE):
        lt = np.asarray(results[c]["out"])
        bc, rc = divmod(c, GRP)
        for hi, ch in enumerate((rc, 7 - rc)):
            full[bc, ch * P : (ch + 1) * P, :] = \
                lt[hi * P : (hi + 1) * P, :] + bout
    return full


def kernel(**inputs):
    in_maps = _make_inputs(**inputs)
    res = run(in_maps)
    return assemble(res.results)


# revision 52
# speedup vs baseline: 1.0487x; 1.0025x over previous
import math
import os
import numpy as np
from contextlib import ExitStack

import concourse.bass as bass
import concourse.tile as tile
from concourse import bacc, mybir
from concourse.bass_utils import run_bass_kernel_spmd
from concourse.masks import make_identity

f32 = mybir.dt.float32
f32r = mybir.dt.float32r
bf16 = mybir.dt.bfloat16
i32 = mybir.dt.int32
AF = mybir.ActivationFunctionType
OP = mybir.AluOpType

V, D, H, L, F = 32000, 768, 12, 6, 3072
B, S = 2, 1024
P = 128
DK = 64
DCH = D // P
FCH = F // P
TOK = 256
NCORE, GRP = 8, 4
VCHK = 500
VBLK = 2000
NVB = V // VBLK
EPS = 1e-5
SQD = math.sqrt(D)
ISQDK = 1.0 / math.sqrt(DK)
VW = DK + 1
VROW = 2 * H * VW

BQ, BK, BV, BO, B2, G1, BE1, G2, BE2, B1 = 0, 6, 12, 18, 24, 30, 36, 42, 48, 54


def build(nc):
    def din(name, shape, dt=f32):
        return nc.dram_tensor(name, shape, dt, kind="ExternalInput").ap()

    tok = din("tok", [P, 2], i32)
    emb = din("emb", [V, D])
    peT = din("peT", [D, TOK])
    maskt = din("maskt", [8 * P, 2 * TOK])
    wq = din("wq", [L, P, DCH * D], bf16)
    wk = din("wk", [L, P, DCH * D], bf16)
    wv = din("wv", [L, P, DCH * D], bf16)
    wo = din("wo", [L, P, DCH * D], bf16)
    w1 = din("w1", [L, P, DCH * F], bf16)
    w2 = din("w2", [L, P, FCH * D], bf16)
    ball = din("ball", [L, P, 78])
    gfp = din("gfp", [P, DCH])
    bfp = din("bfp", [P, DCH])
    woutc = din("woutc", [D, V], bf16)

    out = nc.dram_tensor("out", [TOK, V], f32, kind="ExternalOutput").ap()

    KH = D * TOK // 2
    VH = P * VROW // 2
    kins = [[nc.dram_tensor(f"kin{l}_{p}", [KH], bf16).ap() for p in range(2)]
            for l in range(L)]
    kouts = [[nc.dram_tensor(f"kout{l}_{p}", [GRP * KH], bf16).ap()
              for p in range(2)] for l in range(L)]
    vins = [[nc.dram_tensor(f"vin{l}_{p}", [VH], bf16).ap() for p in range(2)]
            for l in range(L)]
    vouts = [[nc.dram_tensor(f"vout{l}_{p}", [GRP * VH], bf16).ap()
              for p in range(2)] for l in range(L)]
    pr4i = nc.dram_tensor("pr4i", [P], f32).ap()
    pr4o = nc.dram_tensor("pr4o", [GRP * P], f32).ap()

    KV_GROUPS = [[0, 1, 2, 3], [4, 5, 6, 7]]

    with tile.TileContext(
            nc, trace_sim=os.environ.get("TRACE_SIM", "0") == "1",
    ) as tc, ExitStack() as octx, \
            nc.allow_low_precision(reason="bf16 matmul inputs, fp32 accumulate"):
        const = octx.enter_context(tc.tile_pool(name="const", bufs=1))
        acts = octx.enter_context(tc.tile_pool(name="acts", bufs=2))
        stats = octx.enter_context(tc.tile_pool(name="stats", bufs=6))
        psum = octx.enter_context(tc.tile_pool(name="psum", bufs=8, space="PSUM"))

        nc.gpsimd.collective_compute(
            "AllGather", OP.bypass, replica_groups=KV_GROUPS,
            ins=[pr4i.opt()], outs=[pr4o.opt()],
        )

        def ctile(shape, dt, nm):
            return const.tile(shape, dt, name=nm, tag=nm)

        ident = ctile([P, P], f32, "ident")
        make_identity(nc, ident[:])
        ident_b = ctile([P, P], bf16, "ident_b")
        nc.vector.tensor_copy(ident_b[:], ident[:])
        ones_col_f = ctile([P, 1], f32, "ones_col_f")
        nc.vector.memset(ones_col_f[:], 1.0)
        ones_col = ctile([P, 1], f32r, "ones_col")
        nc.vector.tensor_copy(ones_col[:], ones_col_f[:])
        ones_row_f = ctile([1, P], f32, "ones_row_f")
        nc.vector.memset(ones_row_f[:], 1.0)
        ones_row = ctile([1, P], f32r, "ones_row")
        nc.vector.tensor_copy(ones_row[:], ones_row_f[:])
        eps_t = ctile([1, 1], f32, "eps_t")
        nc.vector.memset(eps_t[:], EPS)

        tokt = ctile([P, 2], i32, "tokt")
        nc.sync.dma_start(tokt[:], tok[:])

        mtiles = []
        for kb in range(8):
            mt = ctile([P, 2 * TOK], f32, f"mask{kb}")
            nc.scalar.dma_start(mt[:], maskt[kb * P : (kb + 1) * P, :])
            mtiles.append(mt)

        gft = ctile([P, DCH], f32, "gft")
        nc.sync.dma_start(gft[:], gfp[:])
        bft = ctile([P, DCH], f32, "bft")
        nc.sync.dma_start(bft[:], bfp[:])

        x = [acts.tile([P, TOK], f32r, name=f"x{d}", tag="x", bufs=8)
             for d in range(DCH)]
        xb = [acts.tile([P, TOK], bf16, name=f"xb{d}", tag="xb", bufs=8)
              for d in range(DCH)]
        with ExitStack() as ectx:
            epool = ectx.enter_context(tc.tile_pool(name="epool", bufs=2))
            for j in range(2):
                g = epool.tile([P, D], f32, name="embrow", tag="embrow", bufs=2)
                nc.gpsimd.indirect_dma_start(
                    out=g[:],
                    out_offset=None,
                    in_=emb[:],
                    in_offset=bass.IndirectOffsetOnAxis(ap=tokt[:, j : j + 1], axis=0),
                )
                for dd in range(DCH):
                    pt = epool.tile([P, P], f32, name="pe", tag="pe", bufs=3)
                    nc.sync.dma_start(
                        pt[:], peT[dd * P : (dd + 1) * P, j * P : (j + 1) * P]
                    )
                    ps = psum.tile([P, P], f32, name="tpose", tag="ps")
                    nc.tensor.transpose(ps[:], g[:, dd * P : (dd + 1) * P], ident[:])
                    sl = x[dd][:, j * P : (j + 1) * P]
                    nc.vector.tensor_scalar_mul(sl, ps[:], SQD)
                    nc.vector.tensor_add(sl, sl, pt[:])
        for dd in range(DCH):
            nc.vector.tensor_copy(xb[dd][:], x[dd][:])

        def linear_T(in_b, w_tile, bias_ap, bias_col, out_tile, copy_eng):
            outs = [psum.tile([P, TOK], f32, name=f"lps{oc}", tag="ps")
                    for oc in range(DCH)]
            for ic in range(DCH):
                for oc in range(DCH):
                    nc.tensor.matmul(
                        outs[oc][:],
                        lhsT=w_tile[:, ic * D + oc * P : ic * D + (oc + 1) * P],
                        rhs=in_b[ic][:],
                        start=(ic == 0),
                        stop=(ic == DCH - 1),
                    )
            for oc in range(DCH):
                dst = out_tile[:, oc * TOK : (oc + 1) * TOK]
                bcol = bias_ap[:, bias_col + oc : bias_col + oc + 1]
                if copy_eng == "act":
                    nc.scalar.activation(dst, outs[oc][:], AF.Identity,
                                         bias=bcol)
                else:
                    nc.vector.tensor_scalar_add(dst, outs[oc][:], bcol)

        def ln_T(in_tiles, g_ap, gcol, be_ap, becol, outname, out_tiles=None,
                 outb_tiles=None):
            st_sum = psum.tile([1, TOK], f32, name="lnsum", tag="ps")
            for dd in range(DCH):
                nc.tensor.matmul(
                    st_sum[:], lhsT=ones_col[:], rhs=in_tiles[dd][:],
                    start=(dd == 0), stop=(dd == DCH - 1),
                )
            sqs = []
            for dd in range(DCH):
                sq = acts.tile([P, TOK], f32r, name="lnsq", tag="sq", bufs=4)
                nc.scalar.activation(sq[:], in_tiles[dd][:], AF.Square)
                sqs.append(sq)
            st_sq = psum.tile([1, TOK], f32, name="lnsq2", tag="ps")
            for dd in range(DCH):
                nc.tensor.matmul(
                    st_sq[:], lhsT=ones_col[:], rhs=sqs[dd][:],
                    start=(dd == 0), stop=(dd == DCH - 1),
                )
            nm = stats.tile([1, TOK], f32r, name="nm", tag="st")
            nc.vector.tensor_scalar_mul(nm[:], st_sum[:], -1.0 / D)
            ex2 = stats.tile([1, TOK], f32, name="ex2", tag="st")
            nc.vector.tensor_scalar_mul(ex2[:], st_sq[:], 1.0 / D)
            m2 = stats.tile([1, TOK], f32, name="m2", tag="st")
            nc.vector.tensor_mul(m2[:], nm[:], nm[:])
            var = stats.tile([1, TOK], f32, name="var", tag="st")
            nc.vector.tensor_sub(var[:], ex2[:], m2[:])
            std = stats.tile([1, TOK], f32, name="std", tag="st")
            nc.scalar.activation(std[:], var[:], AF.Sqrt, bias=eps_t[:])
            rstd = stats.tile([1, TOK], f32r, name="rstd", tag="st")
            nc.vector.reciprocal(rstd[:], std[:])
            nmb = psum.tile([P, TOK], f32, name="nmb", tag="ps")
            nc.tensor.matmul(nmb[:], lhsT=ones_row[:], rhs=nm[:],
                             start=True, stop=True)
            rsb = psum.tile([P, TOK], f32, name="rsb", tag="ps")
            nc.tensor.matmul(rsb[:], lhsT=ones_row[:], rhs=rstd[:],
                             start=True, stop=True)
            outs = []
            for dd in range(DCH):
                if out_tiles is None:
                    o = acts.tile([P, TOK], f32r, name=f"{outname}{dd}",
                                  tag="x2", bufs=8)
                else:
                    o = out_tiles[dd]
                nc.vector.tensor_add(o[:], in_tiles[dd][:], nmb[:])
                nc.vector.tensor_mul(o[:], o[:], rsb[:])
                nc.vector.tensor_scalar(
                    o[:], o[:], g_ap[:, gcol + dd : gcol + dd + 1],
                    be_ap[:, becol + dd : becol + dd + 1],
                    op0=OP.mult, op1=OP.add,
                )
                if outb_tiles is not None:
                    nc.scalar.activation(outb_tiles[dd][:], o[:], AF.Identity)
                outs.append(o)
            return outs

        with ExitStack() as lctx:
            wpool = lctx.enter_context(tc.tile_pool(name="wpool", bufs=2))
            wfpool = lctx.enter_context(tc.tile_pool(name="wfpool", bufs=1))
            bpool = lctx.enter_context(tc.tile_pool(name="bpool", bufs=3))
            ktall = lctx.enter_context(tc.tile_pool(name="ktall", bufs=5))
            vall = lctx.enter_context(tc.tile_pool(name="vall", bufs=5))
            expp = lctx.enter_context(tc.tile_pool(name="expp", bufs=6))
            ffp = lctx.enter_context(tc.tile_pool(name="ffp", bufs=3))
            qkvp = lctx.enter_context(tc.tile_pool(name="qkvp", bufs=2))
            opp = lctx.enter_context(tc.tile_pool(name="opp", bufs=2))

            for l in range(L):
                ballt = bpool.tile([P, 78], f32, name="ballt", tag="b", bufs=3)
                nc.sync.dma_start(ballt[:], ball[l])

                wkt = wpool.tile([P, DCH * D], bf16, name="wkt", tag="wk", bufs=1)
                nc.gpsimd.dma_start(wkt[:], wk[l])
                wvt = wpool.tile([P, DCH * D], bf16, name="wvt", tag="wv", bufs=1)
                nc.gpsimd.dma_start(wvt[:], wv[l])
                wqt = wpool.tile([P, DCH * D], bf16, name="wqt", tag="wq", bufs=1)
                nc.gpsimd.dma_start(wqt[:], wq[l])
                wot = wpool.tile([P, DCH * D], bf16, name="wot", tag="wo", bufs=1)
                nc.gpsimd.dma_start(wot[:], wo[l])
                w1t = wfpool.tile([P, DCH * F], bf16, name="w1t", tag="w1", bufs=1)
                nc.gpsimd.dma_start(w1t[:], w1[l])

                KHC = DCH * TOK // 2
                VHC = VROW // 2
                kTt = qkvp.tile([P, DCH * TOK], bf16, name="kTt", tag="k", bufs=1)
                linear_T(xb, wkt, ballt, BK, kTt, "dve")
                for p in range(2):
                    nc.sync.dma_start(
                        kins[l][p].rearrange("(p c) -> p c", p=P),
                        kTt[:, p * KHC : (p + 1) * KHC])
                nc.gpsimd.collective_compute(
                    "AllGather", OP.bypass, replica_groups=KV_GROUPS,
                    ins=[kins[l][0].opt()], outs=[kouts[l][0].opt()],
                )

                vTt = qkvp.tile([P, DCH * TOK], bf16, name="vTt", tag="v", bufs=1)
                linear_T(xb, wvt, ballt, BV, vTt, "act")
                vns = []
                for piece in range(2):
                    vn = vall.tile([P, VHC], bf16, name=f"vn{piece}",
                                   tag="vn", bufs=11)
                    for dd in range(3 * piece, 3 * piece + 3):
                        for tch in range(2):
                            ps = psum.tile([P, P], bf16, name="vtp", tag="ps")
                            nc.tensor.transpose(
                                ps[:],
                                vTt[:, dd * TOK + tch * P :
                                    dd * TOK + (tch + 1) * P],
                                ident_b[:],
                            )
                            for hh in range(2):
                                hm = (2 * dd + hh) % 6
                                nc.vector.tensor_copy(
                                    vn[:, tch * 6 * VW + hm * VW :
                                       tch * 6 * VW + hm * VW + DK],
                                    ps[:, hh * DK : (hh + 1) * DK],
                                )
                    for tch in range(2):
                        for hm in range(6):
                            nc.vector.memset(
                                vn[:, tch * 6 * VW + hm * VW + DK :
                                   tch * 6 * VW + hm * VW + VW], 1.0)
                    vns.append(vn)
                    nc.sync.dma_start(
                        vins[l][piece].rearrange("(p c) -> p c", p=P), vn[:])
                    nc.gpsimd.collective_compute(
                        "AllGather", OP.bypass, replica_groups=KV_GROUPS,
                        ins=[vins[l][piece].opt()],
                        outs=[vouts[l][piece].opt()],
                    )
                    if piece == 0:
                        nc.gpsimd.collective_compute(
                            "AllGather", OP.bypass, replica_groups=KV_GROUPS,
                            ins=[kins[l][1].opt()], outs=[kouts[l][1].opt()],
                        )

                qTt = qkvp.tile([P, DCH * TOK], bf16, name="qTt", tag="q", bufs=1)
                linear_T(xb, wqt, ballt, BQ, qTt, "act")

                KTg = [[], []]
                Vg = [[], []]
                for p in range(2):
                    for g in range(GRP):
                        kt = ktall.tile([P, KHC], bf16, name=f"KT{p}_{g}",
                                        tag="kt", bufs=9)
                        nc.sync.dma_start(
                            kt[:],
                            kouts[l][p][g * KH : (g + 1) * KH]
                            .rearrange("(p c) -> p c", p=P),
                        )
                        KTg[p].append(kt)
                        vg = vall.tile([P, VHC], bf16, name=f"Vg{p}_{g}",
                                       tag="vn", bufs=11)
                        nc.sync.dma_start(
                            vg[:],
                            vouts[l][p][g * VH : (g + 1) * VH]
                            .rearrange("(p c) -> p c", p=P),
                        )
                        Vg[p].append(vg)

                opair = [opp.tile([P, TOK], bf16, name=f"op{hp}", tag="op",
                                  bufs=7) for hp in range(DCH)]
                for hp in range(DCH):
                    pc = hp // 3
                    hpl = hp - 3 * pc
                    oTs = [psum.tile([VW, TOK], f32, name=f"oTps{s}", tag="ps")
                           for s in range(2)]
                    for kb in range(8):
                        g, ch = divmod(kb, 2)
                        for sub in range(2):
                            prow = sub * DK
                            sc = psum.tile([P, TOK], f32, name="scps",
                                           tag="ps")
                            nc.tensor.matmul(
                                sc[:],
                                lhsT=KTg[pc][g][prow : prow + DK,
                                                hpl * TOK + ch * P :
                                                hpl * TOK + (ch + 1) * P],
                                rhs=qTt[prow : prow + DK,
                                        hp * TOK : (hp + 1) * TOK],
                                start=True,
                                stop=True,
                            )
                            e = expp.tile([P, TOK], f32r, name="epre",
                                          tag="ep", bufs=5)
                            nc.vector.scalar_tensor_tensor(
                                e[:], sc[:], ISQDK,
                                mtiles[kb][:, sub * TOK : (sub + 1) * TOK],
                                op0=OP.mult, op1=OP.add)
                            eb = expp.tile([P, TOK], bf16, name="eb",
                                           tag="eb", bufs=5)
                            nc.scalar.activation(eb[:], e[:], AF.Exp)
                            hm = (2 * hp + sub) % 6
                            nc.tensor.matmul(
                                oTs[sub][:],
                                lhsT=Vg[pc][g][:, ch * 6 * VW + hm * VW :
                                               ch * 6 * VW + (hm + 1) * VW],
                                rhs=eb[:],
                                start=(kb == 0),
                                stop=(kb == 7),
                                skip_group_check=True,
                            )
                    for sub in range(2):
                        prow = sub * DK
                        rec = stats.tile([1, TOK], f32r, name="rec", tag="st")
                        nc.vector.reciprocal(rec[:], oTs[sub][DK : DK + 1, :])
                        rb = psum.tile([DK, TOK], f32, name="rbps", tag="ps")
                        nc.tensor.matmul(
                            rb[:], lhsT=ones_row[:, 0:DK], rhs=rec[:],
                            start=True, stop=True, skip_group_check=True,
                        )
                        rbs = acts.tile([DK, TOK], f32, name="rbs", tag="rbs",
                                        bufs=3)
                        nc.vector.tensor_copy(rbs[:], rb[:])
                        nc.vector.tensor_mul(
                            opair[hp][prow : prow + DK, :],
                            oTs[sub][0:DK, :], rbs[:])

                ops_ = [psum.tile([P, TOK], f32, name=f"ops{oc}", tag="ps")
                        for oc in range(DCH)]
                for hpp in range(DCH):
                    for oc in range(DCH):
                        nc.tensor.matmul(
                            ops_[oc][:],
                            lhsT=wot[:, hpp * D + oc * P : hpp * D + (oc + 1) * P],
                            rhs=opair[hpp][:],
                            start=(hpp == 0),
                            stop=(hpp == DCH - 1),
                        )
                t1 = []
                for oc in range(DCH):
                    t = acts.tile([P, TOK], f32r, name=f"t1_{oc}", tag="t",
                                  bufs=8)
                    nc.vector.tensor_scalar_add(t[:], ops_[oc][:],
                                                ballt[:, BO + oc : BO + oc + 1])
                    nc.vector.tensor_add(t[:], t[:], x[oc][:])
                    t1.append(t)
                xn1b = [acts.tile([P, TOK], bf16, name=f"xn1b{d}", tag="xb2",
                                  bufs=8) for d in range(DCH)]
                xn1 = ln_T(t1, ballt, G1, ballt, BE1, "xn1_",
                           outb_tiles=xn1b)

                yps = [psum.tile([P, TOK], f32, name=f"yps{oc}", tag="ps")
                       for oc in range(DCH)]
                for fc0 in range(0, FCH, 2):
                    fpss = []
                    for k in range(2):
                        fpss.append(psum.tile([P, TOK], f32, name=f"ffps{k}",
                                              tag="ps"))
                    for ic in range(DCH):
                        for k in range(2):
                            fc = fc0 + k
                            nc.tensor.matmul(
                                fpss[k][:],
                                lhsT=w1t[:, ic * F + fc * P :
                                         ic * F + (fc + 1) * P],
                                rhs=xn1b[ic][:],
                                start=(ic == 0),
                                stop=(ic == DCH - 1),
                            )
                    for k in range(2):
                        fc = fc0 + k
                        ft = ffp.tile([P, TOK], bf16, name="fft", tag="ff",
                                      bufs=3)
                        if k == 0:
                            nc.scalar.activation(
                                ft[:], fpss[k][:], AF.Relu,
                                bias=ballt[:, B1 + fc : B1 + fc + 1],
                            )
                        else:
                            nc.vector.tensor_scalar(
                                ft[:], fpss[k][:],
                                ballt[:, B1 + fc : B1 + fc + 1], 0.0,
                                op0=OP.add, op1=OP.max,
                            )
                        w2c = wfpool.tile([P, D], bf16, name="w2c", tag="w2",
                                          bufs=8)
                        nc.gpsimd.dma_start(
                            w2c[:], w2[l][:, fc * D : (fc + 1) * D])
                        for oc in range(DCH):
                            nc.tensor.matmul(
                                yps[oc][:],
                                lhsT=w2c[:, oc * P : (oc + 1) * P],
                                rhs=ft[:],
                                start=(fc == 0),
                                stop=(fc == FCH - 1),
                            )
                t2 = []
                for oc in range(DCH):
                    t = acts.tile([P, TOK], f32r, name=f"t2_{oc}", tag="t",
                                  bufs=8)
                    nc.vector.tensor_scalar_add(
                        t[:], yps[oc][:],
                        ballt[:, B2 + oc : B2 + oc + 1])
                    nc.vector.tensor_add(t[:], t[:], xn1[oc][:])
                    t2.append(t)
                x = [acts.tile([P, TOK], f32r, name=f"xo{d}", tag="x",
                               bufs=8) for d in range(DCH)]
                xb = [acts.tile([P, TOK], bf16, name=f"xob{d}", tag="xb",
                                bufs=8) for d in range(DCH)]
                ln_T(t2, ballt, G2, ballt, BE2, f"xl{l}_", out_tiles=x,
                     outb_tiles=xb)

        hb = [acts.tile([P, TOK], bf16, name=f"hb{d}", tag="xb2", bufs=8)
              for d in range(DCH)]
        ln_T(x, gft, 0, bft, 0, "hT_", outb_tiles=hb)

        with ExitStack() as hctx:
            wopool = hctx.enter_context(tc.tile_pool(name="wopool", bufs=2))
            osb = hctx.enter_context(tc.tile_pool(name="osb", bufs=6))

            for vb in range(NVB):
                wts = []
                for dd in range(DCH):
                    wt = wopool.tile([P, VBLK], bf16, name="woutt",
                                     tag=f"wo{dd}", bufs=2)
                    nc.sync.dma_start(
                        wt[:],
                        woutc[dd * P : (dd + 1) * P,
                              vb * VBLK : (vb + 1) * VBLK],
                    )
                    wts.append(wt)
                for tch in range(2):
                    for vc0 in range(0, VBLK // VCHK, 2):
                        lps = [psum.tile([P, VCHK], f32, name=f"logps{k}",
                                         tag="ps") for k in range(2)]
                        for dd in range(DCH):
                            for k in range(2):
                                nc.tensor.matmul(
                                    lps[k][:],
                                    lhsT=hb[dd][:, tch * P : (tch + 1) * P],
                                    rhs=wts[dd][:, (vc0 + k) * VCHK :
                                                (vc0 + k + 1) * VCHK],
                                    start=(dd == 0),
                                    stop=(dd == DCH - 1),
                                )
                        for k in range(2):
                            v0 = vb * VBLK + (vc0 + k) * VCHK
                            ot = osb.tile([P, VCHK], f32, name="lsb",
                                          tag="lsb", bufs=6)
                            if k == 0:
                                nc.vector.tensor_copy(ot[:], lps[k][:])
                            else:
                                nc.scalar.copy(ot[:], lps[k][:])
                            nc.sync.dma_start(
                                out[tch * P : (tch + 1) * P, v0 : v0 + VCHK],
                                ot[:])

    return nc


_CACHED = {}
_BOUT = {}


def _compiled():
    if "nc" not in _CACHED:
        nc = bacc.Bacc("TRN2", target_bir_lowering=False, debug=False,
                       num_devices=NCORE)
        build(nc)
        nc.compile()
        _CACHED["nc"] = nc
    return _CACHED["nc"]


def _make_inputs(tokens, emb, pe, wq, bq, wk, bk, wv, bv, wo, bo,
                 w1, b1, w2, b2, g1, be1, g2, be2, gf, bf, wout, bout):
    import ml_dtypes
    f = np.float32
    b16 = ml_dtypes.bfloat16
    tokens = np.asarray(tokens).astype(np.int32)

    def parr(b):
        b = np.asarray(b, f)
        return b.reshape(L, b.shape[1] // P, P).transpose(0, 2, 1)

    def parr1(b):
        b = np.asarray(b, f)
        return np.ascontiguousarray(b.reshape(b.shape[0] // P, P).T)

    def pslab(w):
        w = np.asarray(w, f)
        Lc, R, C = w.shape
        return np.ascontiguousarray(
            w.reshape(Lc, R // P, P, C).transpose(0, 2, 1, 3)
            .reshape(Lc, P, (R // P) * C).astype(b16))

    ball = np.concatenate(
        [parr(bq), parr(bk), parr(bv), parr(bo), parr(b2),
         parr(g1), parr(be1), parr(g2), parr(be2), parr(b1)], axis=2)

    common = {
        "emb": np.ascontiguousarray(np.asarray(emb, f)),
        "wq": pslab(wq),
        "wk": pslab(wk),
        "wv": pslab(wv),
        "wo": pslab(wo),
        "w1": pslab(w1),
        "w2": pslab(w2),
        "ball": np.ascontiguousarray(ball),
        "gfp": parr1(gf), "bfp": parr1(bf),
        "woutc": np.ascontiguousarray(np.asarray(wout, f).astype(b16)),
    }
    _BOUT["v"] = np.asarray(bout, f)
    pe = np.asarray(pe, f)

    in_maps = []
    for c in range(NCORE):
        b, r = divmod(c, GRP)
        chunks = (r, 7 - r)
        rows = np.concatenate(
            [np.arange(ch * P, (ch + 1) * P) for ch in chunks])
        tok_c = np.stack(
            [tokens[b, ch * P : (ch + 1) * P] for ch in chunks], axis=1
        ).astype(np.int32)
        peT_c = np.ascontiguousarray(pe[rows].T)

        kpos = np.empty(8 * P, np.int64)
        for kb in range(8):
            g, chi = divmod(kb, 2)
            ch = g if chi == 0 else 7 - g
            kpos[kb * P : (kb + 1) * P] = np.arange(ch * P, (ch + 1) * P)
        qpos = rows
        mask = np.where(kpos[:, None] <= qpos[None, :], 0.0, -1e9).astype(f)
        mask2 = np.concatenate([mask, mask], axis=1)

        m = dict(common)
        m.update({
            "tok": tok_c,
            "peT": peT_c,
            "maskt": np.ascontiguousarray(mask2),
        })
        in_maps.append(m)
    return in_maps


def run(in_maps, **kwargs):
    nc = _compiled()
    return run_bass_kernel_spmd(nc, in_maps, list(range(NCORE)), **kwargs)


def assemble(results):
    full = np.empty((B, S, V), np.float32)
    bout = _BOUT["v"]
    for c in range(NCORE):
        lt = np.asarray(results[c]["out"])
        bc, rc = divmod(c, GRP)
        for hi, ch in enumerate((rc, 7 - rc)):
            full[bc, ch * P : (ch + 1) * P, :] = \
                lt[hi * P : (hi + 1) * P, :] + bout
    return full


def kernel(**inputs):
    in_maps = _make_inputs(**inputs)
    res = run(in_maps)
    return assemble(res.results)
